# revision 1
# baseline (speedup 1.0000x reference)
"""Fused GEMM + bias + logsumexp + 2x leaky_relu + 2x exact-gelu for TRN2.

Problem: x:(32768,2048)f16, W:(2048,2048)f16, bias:(2048,)f16
  y = x @ W + bias            (M, N)
  z = logsumexp(y, axis=1)    (M, 1)
  z = leaky_relu(leaky_relu(z, 0.01), 0.01)
  z = gelu(gelu(z, exact))    -> (M, 1) f16

Sharding: data-parallel over M across 8 cores (4096 rows each); W and bias
replicated. logsumexp reduces over N locally, so no cross-core communication.

Per-core structure (~490 us HW, ~89% of the 78.6 TF/s fp16 PE roofline):
- All head copies (bias broadcast, super-block-0 x row-slabs, W in two
  halves) ride the single SWDGE (gpsimd) stream in FIFO order. Tile
  serializes every copy<->transpose transition in the scheduled DMA order,
  so the later DMA-transposes bind to the LAST W half and cannot start early
  and steal HBM bandwidth from W; the HWDGE lanes stay transposes-only.
- Super-block 0's x is transposed ON THE PE (64 [128,128] is_transpose
  matmuls through f16 PSUM, 4 mi-blocks per bank -> one [128,512] DVE
  drain-copy per k) while W streams in — the PE would otherwise idle.
  Splitting W in halves lets m-tile 0's first 8 k-steps start on half 0.
- x super-blocks 1..7 arrive via DMA-transpose (xbar) as 16 per-k tiles
  [128k x 512m], double-buffered, fully hidden under the PE.
- Per 128-row m-tile: 64 matmuls ([128,128]x[128,512] fp16, 16 k-steps x 4
  psum banks), then 4 DVE adds y = psum + bias (f16, matching the
  reference's fp16 GEMM output), a negated row-max reduce, and one ACT Exp
  pass (bias=-max) whose accumulator yields the row sum directly.
- -max / sumexp land in per-m-tile columns of [128, 32] stats tiles; the
  whole logsumexp tail (ln, +max, lrelu^2, erf-based exact gelu^2) runs
  once, batched, at the end — the ACT table stays on Exp for the entire
  main loop instead of thrashing Exp/Ln/Erf per m-tile.
- The [128, 32] result is PE-transposed to [32, 128] so the final store
  writes 256B-contiguous DRAM runs instead of 4096 scattered 2B elements.
"""

import sys
import types

import numpy as np

import concourse.bass as bass
import concourse.tile as tile
from concourse import bacc, mybir
from concourse.bass_utils import run_bass_kernel_spmd
from concourse.masks import make_identity


def _ensure_axon_hooks_stub():
    """bass_utils imports antenv.axon_hooks when BASS_TRACE is set; some
    images lack that module. Provide a no-op stub so a stray env var can't
    crash the run (bass_utils skips tracing when the hook is None)."""
    try:
        import antenv.axon_hooks  # noqa: F401
    except ImportError:
        try:
            import antenv  # noqa: F401
        except ImportError:
            return
        mod = types.ModuleType("antenv.axon_hooks")
        mod._hook = None
        mod.set_axon_ntff_profile_hook = lambda h: setattr(mod, "_hook", h)
        mod.get_axon_ntff_profile_hook = lambda: mod._hook
        sys.modules.setdefault("antenv.axon_hooks", mod)


_ensure_axon_hooks_stub()

M, K, N = 32768, 2048, 2048
N_CORES = 8
M_SHARD = M // N_CORES  # 4096
P = 128
FREE = 512              # matmul moving free dim = one PSUM bank of f32
KT = K // P             # 16 k-subtiles
NB = N // FREE          # 4 psum banks per m-tile

f16 = mybir.dt.float16
f32 = mybir.dt.float32
AF = mybir.ActivationFunctionType
ALU = mybir.AluOpType

SQRT1_2 = 0.7071067811865476
ERF_CLIP = 5.9  # erf(5.9) == 1.0 in fp32; clamp keeps the ACT table in range


def build_program(m_shard=M_SHARD, num_devices=N_CORES):
    nc = bacc.Bacc(
        "TRN2",
        target_bir_lowering=False,
        debug=False,
        enable_asserts=False,
        num_devices=num_devices,
    )
    x = nc.dram_tensor("x", [m_shard, K], f16, kind="ExternalInput").ap()
    W = nc.dram_tensor("W", [K, N], f16, kind="ExternalInput").ap()
    bias = nc.dram_tensor("bias", [N], f16, kind="ExternalInput").ap()
    out = nc.dram_tensor("out", [m_shard, 1], f16, kind="ExternalOutput").ap()

    SBL = 512 if m_shard % 512 == 0 else P  # super-block rows per xT load
    MI = SBL // P                           # m-tiles per super-block
    NSB = m_shard // SBL                    # super-blocks
    MT = m_shard // P                       # total m-tiles

    with tile.TileContext(nc) as tc:
        with (
            tc.tile_pool(name="wpool", bufs=1) as wpool,
            tc.tile_pool(name="xpool", bufs=2) as xpool,
            tc.tile_pool(name="epool", bufs=3) as epool,
            tc.tile_pool(name="spool", bufs=4) as spool,
            tc.tile_pool(name="opool", bufs=1) as opool,
            tc.tile_pool(name="pspool", bufs=8, space="PSUM") as pspool,
        ):
            # ---- head copies: one SWDGE FIFO stream ----
            bias_sb = wpool.tile([P, N], f16, name="bias_sb")
            nc.gpsimd.dma_start(bias_sb[:], bias[None, :].to_broadcast((P, N)))
            xn = []
            for mi in range(MI):
                xnm = xpool.tile([P, K], f16, tag=f"xn{mi}", name=f"xn{mi}")
                nc.gpsimd.dma_start(xnm[:], x[bass.ds(mi * P, P), :])
                xn.append(xnm)

            # identity for PE transposes; must precede the W DMA issues on
            # the gpsimd engine stream (it would otherwise be stranded
            # behind a blocked DMA-issue wait)
            ident = opool.tile([P, P], f16, name="ident")
            make_identity(nc, ident[:])

            # W in two halves (k 0-7 / k 8-15): m-tile 0's early k-steps
            # gate only on the first half. (More pieces measured worse: the
            # scheduler weaves other DMA work between them, and the
            # copy<->transpose mode edges then chain W behind transposes.)
            W_view = W.rearrange("(ko p) n -> p ko n", p=P)
            KH = KT // 2
            Whs = []
            for h in range(2):
                wh = wpool.tile([P, KH, N], f16, tag=f"Wh{h}", name=f"Wh{h}")
                nc.gpsimd.dma_start(wh[:], W_view[:, h * KH : (h + 1) * KH, :])
                Whs.append(wh)

            nm_all = opool.tile([P, MT], f32)  # -rowmax per m-tile column
            se_all = opool.tile([P, MT], f32)  # sum(exp(y-max)) per column

            # ---- PE-transpose super-block 0 while W streams ----
            # Same per-k tiles/tags as the sb1+ DMA-transposes: sharing tags
            # also keeps the scheduler from hoisting sb1's transposes into
            # the head.
            xts = []
            for k in range(KT):
                xk = xpool.tile([P, SBL], f16, tag=f"xk{k}", name=f"xT0_{k}")
                xts.append(xk)
            for k in range(KT):
                # 4 mi-blocks of one k share a PSUM bank -> one [128,512]
                # DVE drain-copy completes the whole xT_k tile
                pt = pspool.tile([P, 2 * FREE], f16, tag="ps", name=f"pt{k}")
                for mi in range(MI):
                    nc.tensor.transpose(
                        pt[:, mi * P : (mi + 1) * P],
                        xn[mi][:, bass.ts(k, P)],
                        ident[:],
                    )
                nc.vector.tensor_copy(xts[k][:], pt[:, : MI * P])

            def issue_transposes(sb):
                xts = []
                for k in range(KT):
                    xk = xpool.tile(
                        [P, SBL], f16, tag=f"xk{k}", name=f"xT{sb}_{k}"
                    )
                    nc.sync.dma_start_transpose(
                        xk[:], x[bass.ds(sb * SBL, SBL), bass.ts(k, P)]
                    )
                    xts.append(xk)
                return xts

            # ---- main loop ----
            for sb in range(NSB):
                if sb > 0:
                    xts = issue_transposes(sb)
                for mi in range(MI):
                    t = sb * MI + mi
                    pss = [
                        pspool.tile([P, FREE], f32, tag="ps", name=f"ps{t}_{nb}")
                        for nb in range(NB)
                    ]
                    for k in range(KT):
                        lhsT = xts[k][:, bass.ts(mi, P)]
                        for nb in range(NB):
                            nc.tensor.matmul(
                                pss[nb][:],
                                lhsT,
                                Whs[k // KH][:, k % KH, bass.ts(nb, FREE)],
                                start=(k == 0),
                                stop=(k == KT - 1),
                            )
                    # y = psum + bias in f16 (the reference's GEMM output is
                    # f16), then negmax = -rowmax(y)
                    y = epool.tile([P, N], f16, tag="yneg", name=f"y{t}")
                    last = t == MT - 1
                    if last:
                        # per-bank row-max for the final m-tile: shortens the
                        # exposed tail after the last matmul
                        mx4 = spool.tile([P, NB], f32, tag="mx4", name="mx4")
                    for nb in range(NB):
                        nc.vector.tensor_tensor(
                            y[:, bass.ts(nb, FREE)],
                            pss[nb][:],
                            bias_sb[:, bass.ts(nb, FREE)],
                            ALU.add,
                        )
                        if last:
                            nc.vector.reduce_max(
                                mx4[:, nb : nb + 1],
                                y[:, bass.ts(nb, FREE)],
                                axis=mybir.AxisListType.X,
                            )
                    if last:
                        nc.vector.reduce_max(
                            nm_all[:, t : t + 1],
                            mx4[:],
                            axis=mybir.AxisListType.X,
                            negate=True,
                        )
                    else:
                        nc.vector.reduce_max(
                            nm_all[:, t : t + 1],
                            y[:, :],
                            axis=mybir.AxisListType.X,
                            negate=True,
                        )
                    # exp(y - max); row-sum via the ACT accumulator
                    ejunk = epool.tile([P, N], f16, tag="ejunk", name=f"ej{t}")
                    nc.scalar.activation(
                        ejunk[:],
                        y[:],
                        AF.Exp,
                        bias=nm_all[:, t : t + 1],
                        accum_out=se_all[:, t : t + 1],
                    )

            # ---- batched tail over all MT m-tiles: [128, MT] ----
            z = opool.tile([P, MT], f32)
            nc.scalar.activation(z[:], se_all[:], AF.Ln)
            nc.vector.tensor_tensor(z[:], z[:], nm_all[:], ALU.subtract)  # +max
            w1 = opool.tile([P, MT], f32)
            for _ in range(2):  # leaky_relu(z, 0.01) = max(z, 0.01 z)
                nc.vector.tensor_scalar_mul(w1[:], z[:], 0.01)
                nc.vector.tensor_tensor(z[:], z[:], w1[:], ALU.max)
            for _ in range(2):  # gelu(z) = 0.5 z (1 + erf(z/sqrt(2)))
                u = opool.tile([P, MT], f32, tag="u")
                nc.vector.tensor_scalar(
                    u[:], z[:], SQRT1_2, ERF_CLIP, ALU.mult, ALU.min
                )
                nc.vector.tensor_scalar_max(u[:], u[:], -ERF_CLIP)
                e = opool.tile([P, MT], f32, tag="e")
                nc.scalar.activation(e[:], u[:], AF.Erf)
                nc.vector.tensor_tensor(e[:], z[:], e[:], ALU.mult)
                nc.vector.tensor_tensor(z[:], z[:], e[:], ALU.add)
                nc.vector.tensor_scalar_mul(z[:], z[:], 0.5)
            z16 = opool.tile([P, MT], f16)
            nc.vector.tensor_copy(z16[:], z[:])

            # PE-transpose [128, MT] -> [MT, 128] (PE is idle by now) so the
            # final store writes 256B-contiguous DRAM runs per partition.
            # Reuses a "ps" slot (same 2KB/partition footprint; all matmul
            # use of the tag is over).
            psT = pspool.tile([MT, 2 * FREE], f16, tag="ps", name="pst")
            nc.tensor.transpose(psT[:, :P], z16[:], ident[:])
            outT = opool.tile([MT, P], f16, name="outT")
            nc.vector.tensor_copy(outT[:], psT[:, :P])
            nc.sync.dma_start(out.rearrange("(t p) o -> t (p o)", p=P), outT[:])

    nc.compile()
    return nc


_prog_cache = {}
LAST_RESULTS = None


def kernel(x, W, bias):
    global LAST_RESULTS
    x = np.ascontiguousarray(x)
    W = np.ascontiguousarray(W)
    bias = np.ascontiguousarray(bias)
    assert x.shape == (M, K) and W.shape == (K, N) and bias.shape == (N,)

    key = (M_SHARD, N_CORES)
    if key not in _prog_cache:
        _prog_cache[key] = build_program(*key)
    nc = _prog_cache[key]

    shards = np.split(x, N_CORES, axis=0)
    in_maps = [{"x": s, "W": W, "bias": bias} for s in shards]
    res = run_bass_kernel_spmd(nc, in_maps, list(range(N_CORES)))
    LAST_RESULTS = res
    return np.concatenate([res.results[i]["out"] for i in range(N_CORES)], axis=0)

